# revision 1
# baseline (speedup 1.0000x reference)
"""Cross-attention Trainium2 kernel, 8 NeuronCores, head-parallel sharding.

Reference computation (fp32):
    q = x @ Wq; k = cond @ Wk; v = cond @ Wv        (per-head dh=40, 8 heads)
    attn = softmax(q k^T / sqrt(dh)); out = (attn v) @ Wo + bo

Sharding: 16 (batch, head) pairs across 8 cores -> core c handles batch c//4,
heads 2*(c%4), 2*(c%4)+1.  Each core computes a partial [S, D_MODEL] output
(its two heads' contribution through Wo); the host sums the 4 partials per
batch and adds the bias.

On-device layout is feature-major ("transposed"):
  - host supplies xT [320, S] and condT [768, SK] (transposed slices)
  - Q^T/K^T [40, S] per head; scores are computed transposed, S^T[keys, q],
    so the exp'd probabilities P^T feed the AV matmul directly as the moving
    operand with V-natural chunks [128 keys, 40] as the stationary operand.
  - an all-ones column appended to V (at 32-aligned index 64) makes the PE
    produce the softmax denominator in the same accumulation; normalization
    happens on the final [40, qb] tile (reciprocal -> ones-outer-product
    broadcast matmul -> elementwise multiply).
All matmuls run in float32r (full PE rate for moving dims >= 256, ~tf32
precision).  Producers of matmul operands must emit float32r-typed outputs.
"""

import sys

for _p in ("/opt/trn_rl_repo", "/root/.axon_site/_ro/trn_rl_repo"):
    if _p not in sys.path:
        sys.path.append(_p)

import numpy as np

B, S, SK = 2, 4096, 4096
D_MODEL, D_COND, H, DH = 320, 768, 8, 40
NCORES = 8
OONE = 64            # ones-column index inside a vaug block (32-aligned)
W = OONE + 1         # vaug block width: [V(40) | zero pad | ones] = 65
QB = 512             # query block (psum bank width in fp32)
KC = 128             # key chunk (psum partitions)
NKC = SK // KC       # 32 key chunks
NQB = S // QB        # 8 query blocks
GRP = 3              # key chunks exp'd per ACT op (3 psum banks wide)
SCALE = DH ** -0.5

_CACHE = {}
_REPEAT_ATTN = 1   # >1: re-run the attention phase (timing calibration only)
_PHASE_LIMIT = 4   # 1=Q only, 2=+K/V, 3=+attention, 4=full (timing attribution)


def _build_nc():
    import concourse.mybir as mybir
    import concourse.tile as tile
    from concourse import bacc
    from concourse.alu_op_type import AluOpType

    F32 = mybir.dt.float32
    F32R = mybir.dt.float32r
    EXP = mybir.ActivationFunctionType.Exp

    nc = bacc.Bacc(None, target_bir_lowering=False)

    xT = nc.dram_tensor("xT", [D_MODEL, S], F32R, kind="ExternalInput")
    condT = nc.dram_tensor("condT", [D_COND, SK], F32R, kind="ExternalInput")
    wq_d = [nc.dram_tensor(f"wq{h}", [D_MODEL, DH], F32R, kind="ExternalInput")
            for h in range(2)]
    wk_d = [nc.dram_tensor(f"wk{h}", [D_COND, DH], F32R, kind="ExternalInput")
            for h in range(2)]
    wv_d = nc.dram_tensor("wv", [D_COND, 2 * DH], F32R, kind="ExternalInput")
    wo_d = [nc.dram_tensor(f"wo{h}", [DH, D_MODEL], F32R, kind="ExternalInput")
            for h in range(2)]
    cpad_d = nc.dram_tensor("cpad", [1, NKC, W], F32R, kind="ExternalInput")
    ones_d = nc.dram_tensor("onesd", [1, DH], F32R, kind="ExternalInput")
    eye_d = nc.dram_tensor("eyed", [DH, DH], F32R, kind="ExternalInput")
    out_d = nc.dram_tensor("out", [S, D_MODEL], F32, kind="ExternalOutput")

    # x/cond feature-dim chunking (128-partition tiles)
    XCH = [(0, 128), (128, 128), (256, 64)]
    CCH = [(c * 128, 128) for c in range(6)]
    CSPLIT = 3           # cond chunks 0..2 staged, 3..5 accumulated + added

    with tile.TileContext(nc) as tc:
      with tc.tile_pool(name="persist", bufs=1) as pp:
        # weights + constants
        wq_t = [[pp.tile([n, DH], F32R, tag=f"wq{h}_{i}", name=f"wq{h}_{i}")
                 for i, (o, n) in enumerate(XCH)] for h in range(2)]
        wk_t = [[pp.tile([128, DH], F32R, tag=f"wk{h}_{i}", name=f"wk{h}_{i}")
                 for i in range(6)] for h in range(2)]
        wv_t = [pp.tile([128, 2 * DH], F32R, tag=f"wv_{i}", name=f"wv_{i}")
                for i in range(6)]
        wo_t = [pp.tile([DH, D_MODEL], F32R, tag=f"wo{h}", name=f"wo{h}")
                for h in range(2)]
        ones_t = pp.tile([1, DH], F32R, tag="ones", name="ones")
        eye_t = pp.tile([DH, DH], F32R, tag="eye", name="eye")
        for h in range(2):
            for i, (o, n) in enumerate(CCH):
                nc.sync.dma_start(wk_t[h][i][:], wk_d[h][o:o + n, :])
        for i, (o, n) in enumerate(CCH):
            nc.sync.dma_start(wv_t[i][:], wv_d[o:o + n, :])
        for h in range(2):
            for i, (o, n) in enumerate(XCH):
                nc.gpsimd.dma_start(wq_t[h][i][:], wq_d[h][o:o + n, :])
            nc.gpsimd.dma_start(wo_t[h][:], wo_d[h][:])
        nc.gpsimd.dma_start(ones_t[:], ones_d[:])
        nc.gpsimd.dma_start(eye_t[:], eye_d[:])

        # persistent activations (K^T, augmented V); Q^T pool opens after cond
        kT = [pp.tile([DH, SK], F32R, tag=f"kT{h}", name=f"kT{h}") for h in range(2)]
        vaug = [pp.tile([128, NKC, W], F32R, tag=f"vaug{h}", name=f"vaug{h}")
                for h in range(2)]
        # vaug init (zeros + per-block ones column) in one broadcast DMA
        pass  # vaug init moved after condT loads

        # ---- phase B: K^T and V (as V^T + PE transpose) from cond ----
        # Contraction is split so PE works while later cond chunks stream in:
        # chunks 0..2 accumulate to a staged psum (ACT-copied to SBUF), chunks
        # 3..5 accumulate to a second psum, DVE adds the two into the result.
        with (
            tc.tile_pool(name="cp", bufs=1) as cp,
            tc.tile_pool(name="vtsb", bufs=1) as vsp,
            tc.tile_pool(name="stg", bufs=3) as stp,
            tc.tile_pool(name="kps", bufs=3, space="PSUM") as kpp,
            tc.tile_pool(name="vtps", bufs=3, space="PSUM") as vpp,
            tc.tile_pool(name="tps", bufs=2, space="PSUM") as tpp,
        ):
            cc = [cp.tile([128, SK], F32R, tag=f"c{i}", name=f"c{i}")
                  for i in range(6)]
            for i, (o, n) in enumerate(CCH):
                nc.sync.dma_start(cc[i][:], condT[o:o + n, :])
            # vaug init (zeros + ones column) — needed only once transposes start
            for h in range(2):
                nc.gpsimd.dma_start(vaug[h][:], cpad_d[:].to_broadcast((128, NKC, W)))

            def split_proj(dst, w_by_chunk, off, tag):
                """dst[:, off:off+QB] = sum_c w_by_chunk[c].T @ cc[c][:, off:off+QB]"""
                ps_a = kpp.tile([DH, QB], F32, tag="kpsa", name="kpsa")
                for i in range(CSPLIT):
                    nc.tensor.matmul(ps_a[:], w_by_chunk[i], cc[i][:, off:off + QB],
                                     start=(i == 0), stop=(i == CSPLIT - 1))
                stg = stp.tile([DH, QB], F32, tag=tag, name=tag)
                nc.scalar.copy(stg[:], ps_a[:])
                ps_b = kpp.tile([DH, QB], F32, tag="kpsb", name="kpsb")
                for i in range(CSPLIT, 6):
                    nc.tensor.matmul(ps_b[:], w_by_chunk[i], cc[i][:, off:off + QB],
                                     start=(i == CSPLIT), stop=(i == 5))
                with nc.allow_low_precision(reason="single f32r rounding of proj"):
                    nc.vector.tensor_tensor(dst, stg[:], ps_b[:], AluOpType.add)

            # V: V^T at full PE rate, then PE-transpose 128-chunks into vaug
            HALF = SK // 2
            for h in range(2):
                for hf in range(2):
                    vts = vsp.tile([DH, HALF], F32R, tag="vts", name="vts")
                    for nb4 in range(HALF // QB):
                        off = hf * HALF + nb4 * QB
                        split_proj(vts[:, nb4 * QB:(nb4 + 1) * QB],
                                   [wv_t[i][:, h * DH:(h + 1) * DH] for i in range(6)],
                                   off, "vstg")
                    for kcl in range(HALF // 128):
                        kt_i = hf * (HALF // 128) + kcl
                        t_ps = tpp.tile([128, DH], F32R, tag="tps", name="tps")
                        nc.tensor.transpose(
                            t_ps[:], vts[:, kcl * 128:(kcl + 1) * 128], eye_t[:])
                        nc.vector.tensor_copy(vaug[h][:, kt_i, 0:DH], t_ps[:])
            for h in range(2):
                for nb in range(NQB):
                    split_proj(kT[h][:, nb * QB:(nb + 1) * QB],
                               [wk_t[h][i][:] for i in range(6)],
                               nb * QB, "kstg")

        # ---- phase A: Q^T = (x @ Wq)^T ----
        qT = [pp.tile([DH, S], F32R, tag=f"qT{h}", name=f"qT{h}") for h in range(2)]
        if _PHASE_LIMIT >= 1:
            with (
                tc.tile_pool(name="xp", bufs=1) as xp,
                tc.tile_pool(name="qps", bufs=4, space="PSUM") as qpp,
            ):
                xc = [xp.tile([n, S], F32R, tag=f"x{i}", name=f"x{i}")
                      for i, (o, n) in enumerate(XCH)]
                xqs = [nc.sync, nc.sync, nc.sync]
                for i, (o, n) in enumerate(XCH):
                    xqs[i].dma_start(xc[i][:], xT[o:o + n, :])
                for h in range(2):
                    for nb in range(NQB):
                        q_ps = qpp.tile([DH, QB], F32, tag="qps", name="qps")
                        # first matmul uses the last-arriving x chunk so the
                        # psum slot is grabbed as late as possible
                        order = [2, 0, 1]
                        for j, i in enumerate(order):
                            nc.tensor.matmul(
                                q_ps[:], wq_t[h][i][:],
                                xc[i][:, nb * QB:(nb + 1) * QB],
                                start=(j == 0), stop=(j == 2),
                            )
                        nc.vector.tensor_copy(qT[h][:, nb * QB:(nb + 1) * QB],
                                              q_ps[:])

        # ---- attention + output projection ----
        groups = [list(range(g, min(g + GRP, NKC))) for g in range(0, NKC, GRP)]
        if _PHASE_LIMIT >= 3:
          with tc.tile_pool(name="ap", bufs=1) as ap:
            outT = [ap.tile([DH, S], F32R, tag=f"outT{h}", name=f"outT{h}")
                    for h in range(2)]
            with (
                tc.tile_pool(name="pt", bufs=3) as ptp,
                tc.tile_pool(name="sc", bufs=2) as scp,
                tc.tile_pool(name="sps", bufs=2, space="PSUM") as spp,
                tc.tile_pool(name="avps", bufs=1, space="PSUM") as avp,
                tc.tile_pool(name="scps", bufs=1, space="PSUM") as scpp,
            ):
              for _rep in range(_REPEAT_ATTN):
                for qb in range(NQB):
                    for h in range(2):
                        q_sl = qT[h][:, qb * QB:(qb + 1) * QB]
                        av = avp.tile([W, QB], F32, tag="av", name="av")
                        for chunks in groups:
                            s_ps = spp.tile([128, GRP * QB], F32, tag="sps",
                                            name="sps")
                            p_t = ptp.tile([128, GRP * QB], F32R, tag="pt",
                                           name="pt")
                            for i, kc in enumerate(chunks):
                                nc.tensor.matmul(
                                    s_ps[:, i * QB:(i + 1) * QB],
                                    kT[h][:, kc * 128:(kc + 1) * 128], q_sl,
                                    start=True, stop=True,
                                )
                            n = len(chunks) * QB
                            nc.scalar.activation(p_t[:, :n], s_ps[:, :n], EXP,
                                                 scale=float(SCALE))
                            for i, kc in enumerate(chunks):
                                nc.tensor.matmul(
                                    av[:], vaug[h][:, kc, :],
                                    p_t[:, i * QB:(i + 1) * QB],
                                    start=(kc == 0), stop=(kc == NKC - 1),
                                )
                        # normalize: av row OONE is sum(exp)
                        recip = scp.tile([1, QB], F32R, tag="recip", name="recip")
                        with nc.allow_low_precision(reason="softmax denominator"):
                            nc.vector.reciprocal(recip[:], av[OONE:OONE + 1, :])
                        sc_ps = scpp.tile([DH, QB], F32, tag="scps", name="scps")
                        nc.tensor.matmul(sc_ps[:], ones_t[:], recip[:],
                                         start=True, stop=True)
                        sc_sb = scp.tile([DH, QB], F32R, tag="scsb", name="scsb")
                        nc.vector.tensor_copy(sc_sb[:], sc_ps[:])
                        with nc.allow_low_precision(reason="normalized attn out"):
                            nc.vector.tensor_mul(
                                outT[h][:, qb * QB:(qb + 1) * QB],
                                av[0:DH, :], sc_sb[:])

            # final projection out[s, :] = sum_h outT_h[:, s]^T @ Wo_h
            if _PHASE_LIMIT >= 4:
                with (
                    tc.tile_pool(name="ops", bufs=6, space="PSUM") as opp,
                    tc.tile_pool(name="ob", bufs=6) as obp,
                ):
                    for st in range(S // 128):
                        o_ps = opp.tile([128, D_MODEL], F32, tag="ops", name="ops")
                        for h in range(2):
                            nc.tensor.matmul(
                                o_ps[:], outT[h][:, st * 128:(st + 1) * 128],
                                wo_t[h][:], start=(h == 0), stop=(h == 1),
                            )
                        o_sb = obp.tile([128, D_MODEL], F32, tag="osb", name="osb")
                        nc.vector.tensor_copy(o_sb[:], o_ps[:])
                        oq = nc.sync if st % 2 == 0 else nc.gpsimd
                        oq.dma_start(out_d[st * 128:(st + 1) * 128, :], o_sb[:])

    nc.compile()
    return nc


def _get_nc():
    if "nc" not in _CACHE:
        _CACHE["nc"] = _build_nc()
    return _CACHE["nc"]


def kernel(x, cond, Wq, Wk, Wv, Wo, bo, _collect_results=None):
    x = np.asarray(x, dtype=np.float32)
    cond = np.asarray(cond, dtype=np.float32)
    Wq = np.asarray(Wq, dtype=np.float32)
    Wk = np.asarray(Wk, dtype=np.float32)
    Wv = np.asarray(Wv, dtype=np.float32)
    Wo = np.asarray(Wo, dtype=np.float32)
    bo = np.asarray(bo, dtype=np.float32)

    from concourse.bass_utils import run_bass_kernel_spmd

    nc = _get_nc()

    cpad = np.zeros((1, NKC, W), dtype=np.float32)
    cpad[0, :, OONE] = 1.0
    ones = np.ones((1, DH), dtype=np.float32)
    eye = np.eye(DH, dtype=np.float32)

    in_maps = []
    for c in range(NCORES):
        b, h0 = c // 4, 2 * (c % 4)
        in_maps.append({
            "xT": np.ascontiguousarray(x[b].T),
            "condT": np.ascontiguousarray(cond[b].T),
            "wq0": np.ascontiguousarray(Wq[:, h0 * DH:(h0 + 1) * DH]),
            "wq1": np.ascontiguousarray(Wq[:, (h0 + 1) * DH:(h0 + 2) * DH]),
            "wk0": np.ascontiguousarray(Wk[:, h0 * DH:(h0 + 1) * DH]),
            "wk1": np.ascontiguousarray(Wk[:, (h0 + 1) * DH:(h0 + 2) * DH]),
            "wv": np.ascontiguousarray(Wv[:, h0 * DH:(h0 + 2) * DH]),
            "wo0": np.ascontiguousarray(Wo[h0 * DH:(h0 + 1) * DH, :]),
            "wo1": np.ascontiguousarray(Wo[(h0 + 1) * DH:(h0 + 2) * DH, :]),
            "cpad": cpad,
            "onesd": ones,
            "eyed": eye,
        })

    kw = _CACHE.pop("run_kwargs", {})
    res = run_bass_kernel_spmd(nc, in_maps, core_ids=list(range(NCORES)), **kw)
    if _collect_results is not None:
        _collect_results.append(res)
    outs = [r["out"] for r in res.results]
    full = np.stack([
        outs[0] + outs[1] + outs[2] + outs[3],
        outs[4] + outs[5] + outs[6] + outs[7],
    ]).astype(np.float32)
    return full + bo[None, None, :]

